# revision 15
# baseline (speedup 1.0000x reference)
"""Causal multi-head attention on 8 trn2 NeuronCores.

Problem: B=2, S=2048, D=1024, H=16 heads, HD=64. fp32 in/out.

Sharding: 8 cores = 2 (batch) x 4 (head groups of 4 heads).
Each core computes, for its batch b and head group g:
  Q^T,K^T  [256, 2048]  (d on partitions, seq on free)  = W^T-slice x
  V        [2048, 256+ones]  (natural, with a ones column per head)
  per 512-wide q chunk, per head:  S^T[k,q] = K^T.T @ Q^T  (PE, contraction 64,
  the two heads of a pair sit on PE row groups 0-63 / 64-127 and their QK
  matmuls are emitted interleaved so the array can run them concurrently),
  P~ = exp(S^T/8) (ACT), causal via block skipping + right-trimming diagonal
  tiles (QK/PV matmuls and exps only cover columns >= 128*dd) + one gpsimd
  affine_select per diagonal 128x128 block, PV: out^T[d,q] accumulated
  over k tiles with V_aug stationary (m=65; row 64 = softmax denominator).
  Reciprocal of den (DVE, read straight out of PSUM row 64), gpsimd
  partition_broadcast, multiply, then O_partial = ctx^T.T @ Wo_rows
  [2048, 1024] (fp16 store).
Host: sums the 4 head-group partials per batch and adds bo + bv @ Wo
(exact: the bv bias contributes the constant row vector bv @ Wo_g).

Wo projections of chunk ci are interleaved into the attention stream of
chunk ci+1 (PE filler while ACT runs exp; spreads the output stores).
Inputs are loaded as ONE merged DMA descriptor per weight matrix / xT
chunk (HWDGE descriptor processing is ~600ns each, so many small
descriptors are queue-bound) split across the SP and ACT hardware DGE
queues; output stores go on the SP queue after its loads.

Default mode "f16": the whole datapath is fp16 (inputs ship as fp16,
matmul accumulation stays fp32 in PSUM; fp16 matmuls run 1 cycle/row at
ANY moving width, unlike float32r which needs >=256). Total rel err vs
the fp32 reference ~1e-3 (budget 2e-2).
"""

import sys

if "/opt/trn_rl_repo" not in sys.path:
    sys.path.insert(0, "/opt/trn_rl_repo")

import numpy as np

import concourse.bacc as bacc
import concourse.bass as bass
import concourse.mybir as mybir
import concourse.tile as tile
from concourse.bass_utils import run_bass_kernel_spmd

B, S, D, H = 2, 2048, 1024, 16
HD = D // H  # 64
N_CORES = 8
HEADS_PER_CORE = H // 4  # 4
DG = HEADS_PER_CORE * HD  # 256 head dims per core
P = 128
CHUNK = 512  # q chunk width
N_KT = S // P  # 16 k tiles
N_CH = S // CHUNK  # 4 q chunks
F32 = mybir.dt.float32
F16 = mybir.dt.float16

_CACHE = {}


def _mm(dt_name):
    return {"f16": mybir.dt.float16,
            "f32r": mybir.dt.float32r, "f16in": mybir.dt.float32r,
            "f32": mybir.dt.float32,
            "bf16": mybir.dt.bfloat16}[dt_name]


def _in_dt(dt_name):
    """dtype for the x / Wq / Wk / Wv inputs (DMA-traffic dominant)."""
    return mybir.dt.float16 if dt_name in ("f16", "f16in") else _mm(dt_name)


def build_kernel(mm_dt="f16", unroll=1, ablate=()):
    """Build + compile the per-core SPMD program. unroll>1 wraps the body
    in a hardware loop (for pure device timing measurements)."""
    mdt = _mm(mm_dt)
    idt = _in_dt(mm_dt)

    nc = bacc.Bacc("TRN2", target_bir_lowering=False, debug=False)
    xT_d = nc.dram_tensor("xT", [D, S], idt, kind="ExternalInput")
    wq_d = nc.dram_tensor("wq", [D, DG], idt, kind="ExternalInput")
    wk_d = nc.dram_tensor("wk", [D, DG], idt, kind="ExternalInput")
    wv_d = nc.dram_tensor("wv", [D, DG], idt, kind="ExternalInput")
    wo_d = nc.dram_tensor("wo", [DG, D], mdt, kind="ExternalInput")
    bq_d = nc.dram_tensor("bq", [DG, 1], F32, kind="ExternalInput")
    bk_d = nc.dram_tensor("bk", [DG, 1], F32, kind="ExternalInput")
    o_d = nc.dram_tensor("o", [S, D], F16, kind="ExternalOutput")

    NDT = D // P  # 8 contraction tiles over D
    NMT = DG // P  # 2 m-tiles over the core's head dims (= head pairs)

    with tile.TileContext(nc) as tc:
        def body(_iv=None):
            _body(tc, nc, mdt, idt,
                  xT_d, wq_d, wk_d, wv_d, wo_d, bq_d, bk_d, o_d, NDT, NMT,
                  ablate)

        if unroll > 1:
            with tc.For_i(0, unroll, 1):
                body()
        else:
            body()

    nc.compile()
    return nc


def _body(tc, nc, mdt, idt, xT_d, wq_d, wk_d, wv_d, wo_d, bq_d, bk_d, o_d,
          NDT, NMT, ablate=()):
    import contextlib
    ctx = contextlib.ExitStack()
    with ctx:
        const = ctx.enter_context(tc.tile_pool(name="const", bufs=1))
        sbuf = ctx.enter_context(tc.tile_pool(name="sbuf", bufs=1))
        ptile_p = ctx.enter_context(tc.tile_pool(name="ptile", bufs=8))
        den_p = ctx.enter_context(tc.tile_pool(name="den", bufs=3))
        out_p = ctx.enter_context(tc.tile_pool(name="outp", bufs=3))
        qkv_ps = ctx.enter_context(
            tc.tile_pool(name="qkv_ps", bufs=2, space="PSUM"))
        stp_ps = ctx.enter_context(
            tc.tile_pool(name="stp_ps", bufs=2, space="PSUM"))
        pv_ps = ctx.enter_context(
            tc.tile_pool(name="pv_ps", bufs=2, space="PSUM"))

        # ---- input tiles (merged: one SBUF tile per tensor) -------------
        # xt_sb: [128, NDT*S]; block k holds rows 128k..128k+127 of xT.
        # w*_sb: [128, NDT*DG]; block k holds rows 128k.. of the weight.
        # wo_sb: [128, NMT*D];  block m holds rows 128m.. of Wo.
        xt_sb = const.tile([P, NDT * S], idt, tag="xt", name="xt")
        w_sb = {}
        for name in ("wq", "wk", "wv"):
            w_sb[name] = const.tile([P, NDT * DG], idt, tag=name, name=name)
        wo_sb = const.tile([P, NMT * D], mdt, tag="wo", name="wo")
        biases = {(name, m): const.tile([P, 1], F32, tag=f"{name}{m}",
                                        name=f"{name}{m}")
                  for name in ("bq", "bk") for m in range(NMT)}

        def xt(k):  # [128, S] view of contraction tile k
            return xt_sb[:, S * k:S * (k + 1)]

        def wslc(name, k):  # [128, DG] view of weight tile k
            return w_sb[name][:, DG * k:DG * (k + 1)]

        # ---- load inputs: one merged descriptor per tensor/chunk --------
        # single sync HWDGE queue, ordered by when each tensor is needed:
        # wv, xt chunk 0 (split in two), wq, wk, xt chunk 1, biases, wo,
        # xt chunks 2-3; the output stores are queued behind these.
        def w_dma(queue, name, d):
            src = d.ap().rearrange("(k p) c -> p k c", p=P)
            dst = w_sb[name][:].rearrange("p (k c) -> p k c", k=NDT)
            queue.dma_start(dst, src)

        xt_src = xT_d.ap().rearrange("(k p) c -> p k c", p=P)
        xt_dst = xt_sb[:].rearrange("p (k c) -> p k c", k=NDT)

        def xt_dma(queue, ci, k0, k1):
            csl = slice(CHUNK * ci, CHUNK * (ci + 1))
            queue.dma_start(xt_dst[:, k0:k1, csl], xt_src[:, k0:k1, csl])

        w_dma(nc.sync, "wv", wv_d)
        xt_dma(nc.scalar, 0, 0, NDT // 2)
        xt_dma(nc.sync, 0, NDT // 2, NDT)
        w_dma(nc.scalar, "wq", wq_d)
        w_dma(nc.sync, "wk", wk_d)
        xt_dma(nc.sync, 1, 0, NDT)
        for (name, m), t in biases.items():
            d = bq_d if name == "bq" else bk_d
            nc.sync.dma_start(t[:], d.ap()[P * m:P * (m + 1), :])
        nc.sync.dma_start(
            wo_sb[:].rearrange("p (m c) -> p m c", m=NMT),
            wo_d.ap().rearrange("(m p) c -> p m c", p=P))
        for ci in range(2, N_CH):
            xt_dma(nc.sync, ci, 0, NDT)

        # ---- V projection (natural layout + ones cols) ------------------
        # vaug[j]: [128, 4*65]; per head h cols h*65..h*65+63 = V, col
        # h*65+64 = 1 (pre-set once; v_proj only writes the V columns)
        ones_f = const.tile([P, HEADS_PER_CORE], F32, tag="ones_f",
                            name="ones_f")
        nc.vector.memset(ones_f[:], 1.0)
        vaug = []
        for j in range(N_KT):
            t = sbuf.tile([P, HEADS_PER_CORE * (HD + 1)], mdt,
                          tag=f"vaug{j}", name=f"vaug{j}")
            vaug.append(t)
            dst = t[:].rearrange("p (h x) -> p h x", h=HEADS_PER_CORE)
            nc.vector.tensor_copy(
                dst[:, :, HD:HD + 1],
                ones_f[:].rearrange("p (h x) -> p h x", x=1))

        def v_proj(j):
            ps = qkv_ps.tile([P, CHUNK], F32, tag="proj", name="proj")
            for k in range(NDT):
                nc.tensor.matmul(
                    ps[:, 0:DG],
                    xt(k)[:, P * j:P * (j + 1)],
                    wslc("wv", k),
                    start=(k == 0), stop=(k == NDT - 1))
            dst = vaug[j][:].rearrange("p (h x) -> p h x", h=HEADS_PER_CORE)
            srcp = ps[:, 0:DG].rearrange("p (h x) -> p h x", h=HEADS_PER_CORE)
            nc.vector.tensor_copy(dst[:, :, 0:HD], srcp[:, :, :])

        # ---- Q^T / K^T projections (d on partitions) --------------------
        qt, kt = [], []
        for name, lst in (("wq", qt), ("wk", kt)):
            for m in range(NMT):
                t = sbuf.tile([P, S], mdt, tag=f"{name}T{m}",
                              name=f"{name}T{m}")
                lst.append(t)

        def qk_one(ci, name, m):
            lst = qt if name == "wq" else kt
            bname = "bq" if name == "wq" else "bk"
            ps = qkv_ps.tile([P, CHUNK], F32, tag="proj", name="proj")
            for k in range(NDT):
                nc.tensor.matmul(
                    ps[:],
                    wslc(name, k)[:, P * m:P * (m + 1)],
                    xt(k)[:, CHUNK * ci:CHUNK * (ci + 1)],
                    start=(k == 0), stop=(k == NDT - 1))
            nc.vector.tensor_scalar_add(
                lst[m][:, CHUNK * ci:CHUNK * (ci + 1)], ps[:],
                biases[(bname, m)][:])

        # ---- Wo projection work units (interleaved into attention) ------
        ctxT = [sbuf.tile([P, S], mdt, tag=f"ctxT{m}", name=f"ctxT{m}")
                for m in range(NMT)]
        wo_ot = {}     # i -> ot tile

        def wo_unit(i, e, tail=False):
            if e == 0:
                wo_ot[i] = out_p.tile([P, D], F16, tag="ot", name="ot")
            ot = wo_ot[i]
            ps = qkv_ps.tile([P, CHUNK], F32, tag="proj", name="proj")
            for m in range(NMT):
                nc.tensor.matmul(
                    ps[:],
                    ctxT[m][:, P * i:P * (i + 1)],
                    wo_sb[:, D * m + CHUNK * e:D * m + CHUNK * (e + 1)],
                    start=(m == 0), stop=(m == NMT - 1))
            dst = ot[:, CHUNK * e:CHUNK * (e + 1)]
            if tail:
                nc.scalar.copy(dst, ps[:])   # ACT is idle at the tail
            else:
                nc.vector.tensor_copy(dst, ps[:])
            if e == 1:
                nc.sync.dma_start(o_d.ap()[P * i:P * (i + 1), :], ot[:])
                del wo_ot[i]

        # ---- PE filler queue: next-chunk projections + Wo units are ----
        # emitted one quantum per attention group (the attention inner
        # loop is ACT-bound, so these fill the PE bubbles)
        from collections import deque
        filler = deque()

        def pump(tail=False):
            if filler:
                filler.popleft()(tail)

        # ---- attention + output projection, per q chunk -----------------
        trim = "notrim" not in ablate

        # chunk 0 projections run up front; later chunks' are fillers
        for j in range(4):
            v_proj(j)
        for name in ("wq", "wk"):
            for m in range(NMT):
                qk_one(0, name, m)

        for ci in range(N_CH):
            if ci + 1 < N_CH:
                nxt = []
                for j in range(4 * (ci + 1), 4 * (ci + 1) + 4):
                    nxt.append(lambda tail, j=j: v_proj(j))
                for name in ("wq", "wk"):
                    for m in range(NMT):
                        nxt.append(
                            lambda tail, ci=ci, name=name, m=m:
                            qk_one(ci + 1, name, m))
                # interleave next-chunk projections with pending Wo units
                old = list(filler)
                filler.clear()
                for a in range(max(len(old), len(nxt))):
                    if a < len(old):
                        filler.append(old[a])
                    if a < len(nxt):
                        filler.append(nxt[a])
            jmax = 4 * ci + 3  # last valid k tile for this chunk
            qsl = slice(CHUNK * ci, CHUNK * (ci + 1))
            for pair in range(NMT):
                pv = [pv_ps.tile([HD + 1, CHUNK], F32, tag="pv", name="pv")
                      for _ in range(2)]
                for j0 in range(0, jmax + 1, 2):
                    js = [j for j in (j0, j0 + 1) if j <= jmax]
                    nj = len(js)
                    # dd[gi]: diagonal offset of k tile js[gi], or None
                    dd = [j - 4 * ci if j >= 4 * ci else None for j in js]
                    # trim start column (within the chunk) per k tile
                    t0 = [P * d if (trim and d) else 0 for d in dd]
                    st = {}
                    pt = {}
                    for hh in range(2):
                        st[hh] = stp_ps.tile([P, 2 * CHUNK], F32, tag="stp",
                                             name="stp")
                        pt[hh] = ptile_p.tile([P, 2 * CHUNK], mdt,
                                              tag="ptile", name="ptile")
                    # QK matmuls, heads interleaved (disjoint PE row groups)
                    for gi, j in enumerate(js):
                        for hh in range(2):
                            psl = slice(64 * hh, 64 * (hh + 1))
                            nc.tensor.matmul(
                                st[hh][:, CHUNK * gi + t0[gi]:
                                       CHUNK * (gi + 1)],
                                kt[pair][psl, P * j:P * (j + 1)],
                                qt[pair][psl,
                                         CHUNK * ci + t0[gi]:
                                         CHUNK * (ci + 1)],
                                start=True, stop=True)
                    # exp (trimmed to the exact written ranges)
                    for hh in range(2):
                        if "exp" in ablate:
                            nc.vector.tensor_copy(
                                pt[hh][:, 0:CHUNK * nj],
                                st[hh][:, 0:CHUNK * nj])
                        elif any(t0):
                            for gi in range(nj):
                                lo = CHUNK * gi + t0[gi]
                                hi = CHUNK * (gi + 1)
                                nc.scalar.activation(
                                    pt[hh][:, lo:hi], st[hh][:, lo:hi],
                                    mybir.ActivationFunctionType.Exp,
                                    scale=0.125)
                        else:
                            nc.scalar.activation(
                                pt[hh][:, 0:CHUNK * nj],
                                st[hh][:, 0:CHUNK * nj],
                                mybir.ActivationFunctionType.Exp,
                                scale=0.125)
                        # causal mask on the diagonal 128x128 block only
                        for gi, j in enumerate(js):
                            if dd[gi] is not None and "mask" not in ablate:
                                base = CHUNK * gi + (P * dd[gi] if trim else 0)
                                w = P if trim else P * (dd[gi] + 1)
                                nc.gpsimd.affine_select(
                                    out=pt[hh][:, base:base + w],
                                    in_=pt[hh][:, base:base + w],
                                    compare_op=mybir.AluOpType.is_ge,
                                    fill=0.0,
                                    base=0 if trim else -P * dd[gi],
                                    pattern=[[1, w]],
                                    channel_multiplier=-1)
                    # PV accumulation (trimmed)
                    for gi, j in enumerate(js):
                        for hh in range(2):
                            h = 2 * pair + hh
                            nc.tensor.matmul(
                                pv[hh][:, t0[gi]:CHUNK],
                                vaug[j][:, (HD + 1) * h:(HD + 1) * (h + 1)],
                                pt[hh][:, CHUNK * gi + t0[gi]:
                                       CHUNK * (gi + 1)],
                                start=(j == 0), stop=(j == jmax))
                    pump()
                # before the last pair's divide, drain leftover fillers of
                # non-final chunks: their PE matmuls overlap the divide and
                # their DVE consumers must precede it (else the PSUM-slot
                # recycle chain stalls PE behind the divide's DVE ops)
                if pair == NMT - 1 and ci + 1 < N_CH:
                    while filler:
                        pump()
                # softmax denominator divide; write ctx^T chunk.
                # reciprocal reads den straight from PSUM row 64 into a
                # base-0 SBUF row (DVE allows the partition shift), gpsimd
                # broadcasts it to 64 partitions, DVE multiplies.
                if "div" in ablate:
                    for hh in range(2):
                        nc.vector.tensor_copy(
                            ctxT[pair][64 * hh:64 * (hh + 1), qsl],
                            pv[hh][0:HD, :])
                else:
                    for hh in range(2):
                        den_t = den_p.tile([1, CHUNK], F32, tag=f"den{hh}",
                                           name=f"den{hh}")
                        nc.vector.reciprocal(den_t[:], pv[hh][HD:HD + 1, :])
                        recb = den_p.tile([HD, CHUNK], F32,
                                          tag=f"recb{hh}", name=f"recb{hh}")
                        nc.gpsimd.partition_broadcast(
                            recb[0:HD, :], den_t[0:1, :])
                        nc.vector.tensor_mul(
                            ctxT[pair][64 * hh:64 * (hh + 1), qsl],
                            pv[hh][0:HD, :],
                            recb[0:HD, :])
            # queue this chunk's Wo work (runs during the next chunk's
            # attention; the last chunk's drains at the tail)
            for i in range(4 * ci, 4 * ci + 4):
                for e in range(2):
                    filler.append(
                        lambda tail, i=i, e=e: wo_unit(i, e, tail=tail))

        while filler:
            pump(tail=True)


def _shard_inputs(x, Wq, bq, Wk, bk, Wv, bv, Wo, bo):
    mm_dt = _CACHE.get("mm_dt", "f16")
    ndt = np.float16 if mm_dt in ("f16", "f16in") else np.float32
    wodt = np.float16 if mm_dt == "f16" else np.float32
    x = np.asarray(x, np.float32)
    in_maps = []
    for core in range(N_CORES):
        b, g = divmod(core, 4)
        ds = slice(DG * g, DG * (g + 1))
        in_maps.append({
            "xT": np.ascontiguousarray(x[b].T).astype(ndt),
            "wq": np.ascontiguousarray(
                np.asarray(Wq, np.float32)[:, ds]).astype(ndt),
            "wk": np.ascontiguousarray(
                np.asarray(Wk, np.float32)[:, ds]).astype(ndt),
            "wv": np.ascontiguousarray(
                np.asarray(Wv, np.float32)[:, ds]).astype(ndt),
            "wo": np.ascontiguousarray(
                np.asarray(Wo, np.float32)[ds, :]).astype(wodt),
            "bq": np.asarray(bq, np.float32)[ds].reshape(DG, 1).copy(),
            "bk": np.asarray(bk, np.float32)[ds].reshape(DG, 1).copy(),
        })
    return in_maps


def kernel(x, Wq, bq, Wk, bk, Wv, bv, Wo, bo):
    mm_dt = _CACHE.get("mm_dt", "f16")
    _CACHE["mm_dt"] = mm_dt
    if "nc" not in _CACHE:
        _CACHE["nc"] = build_kernel(mm_dt)
    nc = _CACHE["nc"]
    in_maps = _shard_inputs(x, Wq, bq, Wk, bk, Wv, bv, Wo, bo)
    res = run_bass_kernel_spmd(
        nc, in_maps, core_ids=list(range(N_CORES)), trace=False)
    out = np.zeros((B, S, D), np.float32)
    for core in range(N_CORES):
        out[core // 4] += res.results[core]["o"]
    # exact bias folding: +bo, + bv @ Wo (constant row vector)
    out += (np.asarray(bo, np.float32)
            + np.asarray(bv, np.float32) @ np.asarray(Wo, np.float32))
    return out


# revision 17
# speedup vs baseline: 1.0263x; 1.0263x over previous
"""Causal multi-head attention on 8 trn2 NeuronCores.

Problem: B=2, S=2048, D=1024, H=16 heads, HD=64. fp32 in/out.

Sharding: 8 cores = 2 (batch) x 4 (head groups of 4 heads).
Each core computes, for its batch b and head group g:
  Q^T,K^T  [256, 2048]  (d on partitions, seq on free)  = W^T-slice x
  V        [2048, 256+ones]  (natural, with a ones column per head)
  per 512-wide q chunk, per head:  S^T[k,q] = K^T.T @ Q^T  (PE, contraction 64,
  the two heads of a pair sit on PE row groups 0-63 / 64-127 and their QK
  matmuls are emitted interleaved so the array can run them concurrently),
  P~ = exp(S^T/8) (ACT), causal via block skipping + right-trimming diagonal
  tiles (QK/PV matmuls and exps only cover columns >= 128*dd) + one gpsimd
  affine_select per diagonal 128x128 block, PV: out^T[d,q] accumulated
  over k tiles with V_aug stationary (m=65; row 64 = softmax denominator).
  Reciprocal of den (DVE, read straight out of PSUM row 64), gpsimd
  partition_broadcast, multiply, then O_partial = ctx^T.T @ Wo_rows
  [2048, 1024] (fp16 store).
Host: sums the 4 head-group partials per batch and adds bo + bv @ Wo
(exact: the bv bias contributes the constant row vector bv @ Wo_g).

Wo projections of chunk ci are interleaved into the attention stream of
chunk ci+1 (PE filler while ACT runs exp; spreads the output stores).
Inputs are loaded as ONE merged DMA descriptor per weight matrix / xT
chunk (HWDGE descriptor processing is ~600ns each, so many small
descriptors are queue-bound) split across the SP and ACT hardware DGE
queues; output stores go on the SP queue after its loads.

Default mode "f16": the whole datapath is fp16 (inputs ship as fp16,
matmul accumulation stays fp32 in PSUM; fp16 matmuls run 1 cycle/row at
ANY moving width, unlike float32r which needs >=256). Total rel err vs
the fp32 reference ~1e-3 (budget 2e-2).
"""

import sys

if "/opt/trn_rl_repo" not in sys.path:
    sys.path.insert(0, "/opt/trn_rl_repo")

import numpy as np

import concourse.bacc as bacc
import concourse.bass as bass
import concourse.mybir as mybir
import concourse.tile as tile
from concourse.bass_utils import run_bass_kernel_spmd

B, S, D, H = 2, 2048, 1024, 16
HD = D // H  # 64
N_CORES = 8
HEADS_PER_CORE = H // 4  # 4
DG = HEADS_PER_CORE * HD  # 256 head dims per core
P = 128
CHUNK = 512  # q chunk width
N_KT = S // P  # 16 k tiles
N_CH = S // CHUNK  # 4 q chunks
F32 = mybir.dt.float32
F16 = mybir.dt.float16

_CACHE = {}


def _mm(dt_name):
    return {"f16": mybir.dt.float16,
            "f32r": mybir.dt.float32r, "f16in": mybir.dt.float32r,
            "f32": mybir.dt.float32,
            "bf16": mybir.dt.bfloat16}[dt_name]


def _in_dt(dt_name):
    """dtype for the x / Wq / Wk / Wv inputs (DMA-traffic dominant)."""
    return mybir.dt.float16 if dt_name in ("f16", "f16in") else _mm(dt_name)


def build_kernel(mm_dt="f16", unroll=1, ablate=()):
    """Build + compile the per-core SPMD program. unroll>1 wraps the body
    in a hardware loop (for pure device timing measurements)."""
    mdt = _mm(mm_dt)
    idt = _in_dt(mm_dt)

    nc = bacc.Bacc("TRN2", target_bir_lowering=False, debug=False)
    xT_d = nc.dram_tensor("xT", [D, S], idt, kind="ExternalInput")
    wq_d = nc.dram_tensor("wq", [D, DG], idt, kind="ExternalInput")
    wk_d = nc.dram_tensor("wk", [D, DG], idt, kind="ExternalInput")
    wv_d = nc.dram_tensor("wv", [D, DG], idt, kind="ExternalInput")
    wo_d = nc.dram_tensor("wo", [DG, D], mdt, kind="ExternalInput")
    bq_d = nc.dram_tensor("bq", [DG, 1], F32, kind="ExternalInput")
    bk_d = nc.dram_tensor("bk", [DG, 1], F32, kind="ExternalInput")
    o_d = nc.dram_tensor("o", [S, D], F16, kind="ExternalOutput")

    NDT = D // P  # 8 contraction tiles over D
    NMT = DG // P  # 2 m-tiles over the core's head dims (= head pairs)

    with tile.TileContext(nc) as tc:
        def body(_iv=None):
            _body(tc, nc, mdt, idt,
                  xT_d, wq_d, wk_d, wv_d, wo_d, bq_d, bk_d, o_d, NDT, NMT,
                  ablate)

        if unroll > 1:
            with tc.For_i(0, unroll, 1):
                body()
        else:
            body()

    nc.compile()
    return nc


def _body(tc, nc, mdt, idt, xT_d, wq_d, wk_d, wv_d, wo_d, bq_d, bk_d, o_d,
          NDT, NMT, ablate=()):
    import contextlib
    ctx = contextlib.ExitStack()
    with ctx:
        const = ctx.enter_context(tc.tile_pool(name="const", bufs=1))
        sbuf = ctx.enter_context(tc.tile_pool(name="sbuf", bufs=1))
        ptile_p = ctx.enter_context(tc.tile_pool(name="ptile", bufs=8))
        den_p = ctx.enter_context(tc.tile_pool(name="den", bufs=3))
        out_p = ctx.enter_context(tc.tile_pool(name="outp", bufs=3))
        qkv_ps = ctx.enter_context(
            tc.tile_pool(name="qkv_ps", bufs=2, space="PSUM"))
        stp_ps = ctx.enter_context(
            tc.tile_pool(name="stp_ps", bufs=2, space="PSUM"))
        pv_ps = ctx.enter_context(
            tc.tile_pool(name="pv_ps", bufs=2, space="PSUM"))

        # ---- input tiles (merged: one SBUF tile per tensor) -------------
        # xt_sb: [128, NDT*S]; block k holds rows 128k..128k+127 of xT.
        # w*_sb: [128, NDT*DG]; block k holds rows 128k.. of the weight.
        # wo_sb: [128, NMT*D];  block m holds rows 128m.. of Wo.
        xt_sb = const.tile([P, NDT * S], idt, tag="xt", name="xt")
        w_sb = {}
        for name in ("wq", "wk", "wv"):
            w_sb[name] = const.tile([P, NDT * DG], idt, tag=name, name=name)
        wo_sb = const.tile([P, NMT * D], mdt, tag="wo", name="wo")
        biases = {(name, m): const.tile([P, 1], F32, tag=f"{name}{m}",
                                        name=f"{name}{m}")
                  for name in ("bq", "bk") for m in range(NMT)}

        def xt(k):  # [128, S] view of contraction tile k
            return xt_sb[:, S * k:S * (k + 1)]

        def wslc(name, k):  # [128, DG] view of weight tile k
            return w_sb[name][:, DG * k:DG * (k + 1)]

        # ---- load inputs: one merged descriptor per tensor/chunk --------
        # single sync HWDGE queue, ordered by when each tensor is needed:
        # wv, xt chunk 0 (split in two), wq, wk, xt chunk 1, biases, wo,
        # xt chunks 2-3; the output stores are queued behind these.
        def w_dma(queue, name, d):
            src = d.ap().rearrange("(k p) c -> p k c", p=P)
            dst = w_sb[name][:].rearrange("p (k c) -> p k c", k=NDT)
            queue.dma_start(dst, src)

        xt_src = xT_d.ap().rearrange("(k p) c -> p k c", p=P)
        xt_dst = xt_sb[:].rearrange("p (k c) -> p k c", k=NDT)

        def xt_dma(queue, ci, k0, k1):
            csl = slice(CHUNK * ci, CHUNK * (ci + 1))
            queue.dma_start(xt_dst[:, k0:k1, csl], xt_src[:, k0:k1, csl])

        w_dma(nc.sync, "wv", wv_d)
        xt_dma(nc.scalar, 0, 0, NDT // 2)
        xt_dma(nc.sync, 0, NDT // 2, NDT)
        w_dma(nc.scalar, "wq", wq_d)
        w_dma(nc.sync, "wk", wk_d)
        xt_dma(nc.sync, 1, 0, NDT)
        for (name, m), t in biases.items():
            d = bq_d if name == "bq" else bk_d
            nc.sync.dma_start(t[:], d.ap()[P * m:P * (m + 1), :])
        nc.sync.dma_start(
            wo_sb[:].rearrange("p (m c) -> p m c", m=NMT),
            wo_d.ap().rearrange("(m p) c -> p m c", p=P))
        for ci in range(2, N_CH):
            xt_dma(nc.sync, ci, 0, NDT)

        # ---- V projection (natural layout + ones cols) ------------------
        # vaug[j]: [128, 4*65]; per head h cols h*65..h*65+63 = V, col
        # h*65+64 = 1 (pre-set once; v_proj only writes the V columns)
        ones_f = const.tile([P, HEADS_PER_CORE], F32, tag="ones_f",
                            name="ones_f")
        nc.vector.memset(ones_f[:], 1.0)
        vaug = []
        for j in range(N_KT):
            t = sbuf.tile([P, HEADS_PER_CORE * (HD + 1)], mdt,
                          tag=f"vaug{j}", name=f"vaug{j}")
            vaug.append(t)
            dst = t[:].rearrange("p (h x) -> p h x", h=HEADS_PER_CORE)
            nc.vector.tensor_copy(
                dst[:, :, HD:HD + 1],
                ones_f[:].rearrange("p (h x) -> p h x", x=1))

        def v_proj(j):
            ps = qkv_ps.tile([P, CHUNK], F32, tag="proj", name="proj")
            for k in range(NDT):
                nc.tensor.matmul(
                    ps[:, 0:DG],
                    xt(k)[:, P * j:P * (j + 1)],
                    wslc("wv", k),
                    start=(k == 0), stop=(k == NDT - 1))
            dst = vaug[j][:].rearrange("p (h x) -> p h x", h=HEADS_PER_CORE)
            srcp = ps[:, 0:DG].rearrange("p (h x) -> p h x", h=HEADS_PER_CORE)
            nc.vector.tensor_copy(dst[:, :, 0:HD], srcp[:, :, :])

        # ---- Q^T / K^T projections (d on partitions) --------------------
        qt, kt = [], []
        for name, lst in (("wq", qt), ("wk", kt)):
            for m in range(NMT):
                t = sbuf.tile([P, S], mdt, tag=f"{name}T{m}",
                              name=f"{name}T{m}")
                lst.append(t)

        def qk_one(ci, name, m):
            lst = qt if name == "wq" else kt
            bname = "bq" if name == "wq" else "bk"
            ps = qkv_ps.tile([P, CHUNK], F32, tag="proj", name="proj")
            for k in range(NDT):
                nc.tensor.matmul(
                    ps[:],
                    wslc(name, k)[:, P * m:P * (m + 1)],
                    xt(k)[:, CHUNK * ci:CHUNK * (ci + 1)],
                    start=(k == 0), stop=(k == NDT - 1))
            nc.vector.tensor_scalar_add(
                lst[m][:, CHUNK * ci:CHUNK * (ci + 1)], ps[:],
                biases[(bname, m)][:])

        # ---- Wo projection work units (interleaved into attention) ------
        ctxT = [sbuf.tile([P, S], mdt, tag=f"ctxT{m}", name=f"ctxT{m}")
                for m in range(NMT)]
        wo_ot = {}     # i -> ot tile

        def wo_unit(i, e, tail=False):
            if e == 0:
                wo_ot[i] = out_p.tile([P, D], F16, tag="ot", name="ot")
            ot = wo_ot[i]
            ps = qkv_ps.tile([P, CHUNK], F32, tag="proj", name="proj")
            for m in range(NMT):
                nc.tensor.matmul(
                    ps[:],
                    ctxT[m][:, P * i:P * (i + 1)],
                    wo_sb[:, D * m + CHUNK * e:D * m + CHUNK * (e + 1)],
                    start=(m == 0), stop=(m == NMT - 1))
            dst = ot[:, CHUNK * e:CHUNK * (e + 1)]
            if tail:
                nc.scalar.copy(dst, ps[:])   # ACT is idle at the tail
            else:
                nc.vector.tensor_copy(dst, ps[:])
            if e == 1:
                nc.sync.dma_start(o_d.ap()[P * i:P * (i + 1), :], ot[:])
                del wo_ot[i]

        # ---- PE filler queue: next-chunk projections + Wo units are ----
        # emitted one quantum per attention group (the attention inner
        # loop is ACT-bound, so these fill the PE bubbles)
        from collections import deque
        filler = deque()

        def pump(tail=False):
            if filler:
                filler.popleft()(tail)

        # ---- attention + output projection, per q chunk -----------------
        trim = "notrim" not in ablate

        # chunk 0 projections run up front; later chunks' are fillers
        for j in range(4):
            v_proj(j)
        for name in ("wq", "wk"):
            for m in range(NMT):
                qk_one(0, name, m)

        for ci in range(N_CH):
            if ci + 1 < N_CH:
                nxt = []
                for j in range(4 * (ci + 1), 4 * (ci + 1) + 4):
                    nxt.append(lambda tail, j=j: v_proj(j))
                for name in ("wq", "wk"):
                    for m in range(NMT):
                        nxt.append(
                            lambda tail, ci=ci, name=name, m=m:
                            qk_one(ci + 1, name, m))
                # interleave next-chunk projections with pending Wo units
                old = list(filler)
                filler.clear()
                for a in range(max(len(old), len(nxt))):
                    if a < len(old):
                        filler.append(old[a])
                    if a < len(nxt):
                        filler.append(nxt[a])
            jmax = 4 * ci + 3  # last valid k tile for this chunk
            qsl = slice(CHUNK * ci, CHUNK * (ci + 1))
            for pair in range(NMT):
                pv = [pv_ps.tile([HD + 1, CHUNK], F32, tag="pv", name="pv")
                      for _ in range(2)]
                for j0 in range(0, jmax + 1, 2):
                    js = [j for j in (j0, j0 + 1) if j <= jmax]
                    nj = len(js)
                    # dd[gi]: diagonal offset of k tile js[gi], or None
                    dd = [j - 4 * ci if j >= 4 * ci else None for j in js]
                    # trim start column (within the chunk) per k tile
                    t0 = [P * d if (trim and d) else 0 for d in dd]
                    st = {}
                    pt = {}
                    for hh in range(2):
                        st[hh] = stp_ps.tile([P, 2 * CHUNK], F32, tag="stp",
                                             name="stp")
                        pt[hh] = ptile_p.tile([P, 2 * CHUNK], mdt,
                                              tag="ptile", name="ptile")
                    # QK matmuls, heads interleaved (disjoint PE row groups)
                    for gi, j in enumerate(js):
                        for hh in range(2):
                            psl = slice(64 * hh, 64 * (hh + 1))
                            nc.tensor.matmul(
                                st[hh][:, CHUNK * gi + t0[gi]:
                                       CHUNK * (gi + 1)],
                                kt[pair][psl, P * j:P * (j + 1)],
                                qt[pair][psl,
                                         CHUNK * ci + t0[gi]:
                                         CHUNK * (ci + 1)],
                                start=True, stop=True)
                    # exp (trimmed to the exact written ranges)
                    for hh in range(2):
                        if "exp" in ablate:
                            nc.vector.tensor_copy(
                                pt[hh][:, 0:CHUNK * nj],
                                st[hh][:, 0:CHUNK * nj])
                        elif any(t0):
                            for gi in range(nj):
                                lo = CHUNK * gi + t0[gi]
                                hi = CHUNK * (gi + 1)
                                nc.scalar.activation(
                                    pt[hh][:, lo:hi], st[hh][:, lo:hi],
                                    mybir.ActivationFunctionType.Exp,
                                    scale=0.125)
                        else:
                            nc.scalar.activation(
                                pt[hh][:, 0:CHUNK * nj],
                                st[hh][:, 0:CHUNK * nj],
                                mybir.ActivationFunctionType.Exp,
                                scale=0.125)
                        # causal mask on the diagonal 128x128 block only
                        for gi, j in enumerate(js):
                            if dd[gi] is not None and "mask" not in ablate:
                                base = CHUNK * gi + (P * dd[gi] if trim else 0)
                                w = P if trim else P * (dd[gi] + 1)
                                nc.gpsimd.affine_select(
                                    out=pt[hh][:, base:base + w],
                                    in_=pt[hh][:, base:base + w],
                                    compare_op=mybir.AluOpType.is_ge,
                                    fill=0.0,
                                    base=0 if trim else -P * dd[gi],
                                    pattern=[[1, w]],
                                    channel_multiplier=-1)
                    # PV accumulation (trimmed)
                    for gi, j in enumerate(js):
                        for hh in range(2):
                            h = 2 * pair + hh
                            nc.tensor.matmul(
                                pv[hh][:, t0[gi]:CHUNK],
                                vaug[j][:, (HD + 1) * h:(HD + 1) * (h + 1)],
                                pt[hh][:, CHUNK * gi + t0[gi]:
                                       CHUNK * (gi + 1)],
                                start=(j == 0), stop=(j == jmax))
                    pump()
                # before the last pair's divide, drain leftover fillers of
                # non-final chunks: their PE matmuls overlap the divide and
                # their DVE consumers must precede it (else the PSUM-slot
                # recycle chain stalls PE behind the divide's DVE ops)
                if pair == NMT - 1 and ci + 1 < N_CH:
                    while filler:
                        pump()
                # softmax denominator divide; write ctx^T chunk.
                # reciprocal reads den straight from PSUM row 64 into a
                # base-0 SBUF row (DVE allows the partition shift), gpsimd
                # broadcasts it to 64 partitions, DVE multiplies.
                if "div" in ablate:
                    for hh in range(2):
                        nc.vector.tensor_copy(
                            ctxT[pair][64 * hh:64 * (hh + 1), qsl],
                            pv[hh][0:HD, :])
                else:
                    for hh in range(2):
                        den_t = den_p.tile([1, CHUNK], F32, tag=f"den{hh}",
                                           name=f"den{hh}")
                        nc.vector.reciprocal(den_t[:], pv[hh][HD:HD + 1, :])
                        recb = den_p.tile([HD, CHUNK], F32,
                                          tag=f"recb{hh}", name=f"recb{hh}")
                        nc.gpsimd.partition_broadcast(
                            recb[0:HD, :], den_t[0:1, :])
                        nc.vector.tensor_mul(
                            ctxT[pair][64 * hh:64 * (hh + 1), qsl],
                            pv[hh][0:HD, :],
                            recb[0:HD, :])
            # queue this chunk's Wo work (runs during the next chunk's
            # attention; the last chunk's drains at the tail)
            for i in range(4 * ci, 4 * ci + 4):
                for e in range(2):
                    filler.append(
                        lambda tail, i=i, e=e: wo_unit(i, e, tail=tail))

        while filler:
            pump(tail=True)


def _shard_inputs(x, Wq, bq, Wk, bk, Wv, bv, Wo, bo):
    mm_dt = _CACHE.get("mm_dt", "f16")
    ndt = np.float16 if mm_dt in ("f16", "f16in") else np.float32
    wodt = np.float16 if mm_dt == "f16" else np.float32
    x = np.asarray(x, np.float32)
    in_maps = []
    for core in range(N_CORES):
        b, g = divmod(core, 4)
        ds = slice(DG * g, DG * (g + 1))
        in_maps.append({
            "xT": np.ascontiguousarray(x[b].T).astype(ndt),
            "wq": np.ascontiguousarray(
                np.asarray(Wq, np.float32)[:, ds]).astype(ndt),
            "wk": np.ascontiguousarray(
                np.asarray(Wk, np.float32)[:, ds]).astype(ndt),
            "wv": np.ascontiguousarray(
                np.asarray(Wv, np.float32)[:, ds]).astype(ndt),
            "wo": np.ascontiguousarray(
                np.asarray(Wo, np.float32)[ds, :]).astype(wodt),
            "bq": np.asarray(bq, np.float32)[ds].reshape(DG, 1).copy(),
            "bk": np.asarray(bk, np.float32)[ds].reshape(DG, 1).copy(),
        })
    return in_maps


def kernel(x, Wq, bq, Wk, bk, Wv, bv, Wo, bo):
    mm_dt = _CACHE.get("mm_dt", "f16")
    _CACHE["mm_dt"] = mm_dt
    if "nc" not in _CACHE:
        _CACHE["nc"] = build_kernel(mm_dt)
    nc = _CACHE["nc"]
    in_maps = _shard_inputs(x, Wq, bq, Wk, bk, Wv, bv, Wo, bo)
    res = run_bass_kernel_spmd(
        nc, in_maps, core_ids=list(range(N_CORES)), trace=False)
    out = np.zeros((B, S, D), np.float32)
    for core in range(N_CORES):
        out[core // 4] += res.results[core]["o"]
    # exact bias folding: +bo, + bv @ Wo (constant row vector)
    out += (np.asarray(bo, np.float32)
            + np.asarray(bv, np.float32) @ np.asarray(Wo, np.float32))
    return out
